# revision 5
# baseline (speedup 1.0000x reference)
"""Multi-head attention (B=2, S=4096, D=512, H=8) on 8 TRN2 NeuronCores.

Sharding: batch x query-block. Core c handles batch b=c//4, query rows
[q0, q0+1024) with q0=(c%4)*1024, all 8 heads, full key range.

Everything on-device is computed in "transposed" orientation:
  scores^T [sk, sq] = kT_h^T-stationary @ qT_h-moving
  Em^T = exp(scores^T) * (1-mask)^T          (ACT exp + DVE multiply)
  ctx^T [65, sq] = sum_sk v_aug^T-stationary @ Em^T-moving
     (v_aug has a ones column per head -> row 64 of ctx^T = softmax rowsum)
  P^T = Em^T * broadcast(1/rowsum)           -> DRAM as attnT [8, 4096, 1024]
  out  = ctx_n^T-stationary @ Wo-moving + bo -> DRAM [1024, 512]
Host prep uploads Q/K/V/W pre-transposed & fp16; host gather un-transposes
attnT into attn_prob[b, h, sq, sk].
"""
import numpy as np

import concourse.bass as bass
import concourse.tile as tile
from concourse import bacc, mybir
from concourse.bass_utils import run_bass_kernel_spmd
from contextlib import ExitStack

F32 = mybir.dt.float32
F16 = mybir.dt.float16

B, S, D = 2, 4096, 512
H, DH = 8, 64
DHA = DH + 1          # ones column appended per head
SQ = 1024             # query rows per core
SQB = 512             # query block (2 per core)
NSKT = S // 128       # 32 sk tiles
N_CORES = 8
EXP = mybir.ActivationFunctionType.Exp
MULT = mybir.AluOpType.mult

_BUILD_CACHE = {}


def build(repeat: int = 1):
    if repeat in _BUILD_CACHE:
        return _BUILD_CACHE[repeat]
    nc = bacc.Bacc("TRN2", target_bir_lowering=False, debug=False,
                   num_devices=N_CORES)

    def inp(name, shape):
        return nc.dram_tensor(name, shape, F16, kind="ExternalInput").ap()

    dQT = inp("QT", [D, SQ])
    dKT = inp("KT", [D, S])
    dVT = inp("VT", [D, S])
    dMT = inp("mbarT", [S, SQ])
    dWq = inp("Wq", [D, D])
    dWk = inp("Wk", [D, D])
    dWva = inp("Wva", [D, H * DHA])
    dWo = inp("Wo", [D, D])
    dbq = inp("bq", [1, D])
    dbk = inp("bk", [1, D])
    dbva = inp("bva", [1, H * DHA])
    dbo = inp("bo", [1, D])
    dones = inp("ones", [1, SQ])

    dAttnT = nc.dram_tensor("attnT", [H, S, SQ], F32,
                            kind="ExternalOutput").ap()
    dOut = nc.dram_tensor("outp", [SQ, D], F32, kind="ExternalOutput").ap()

    with tile.TileContext(nc) as tc, ExitStack() as ctx:
        # ---- persistent pools ----
        cst = ctx.enter_context(tc.tile_pool(name="cst", bufs=1))
        proj = ctx.enter_context(tc.tile_pool(name="proj", bufs=1))

        t_ones = cst.tile([1, SQ], F16, name="t_ones")
        nc.sync.dma_start(t_ones[:], dones)
        t_bq = cst.tile([1, D], F16, name="t_bq")
        nc.sync.dma_start(t_bq[:], dbq)
        t_bk = cst.tile([1, D], F16, name="t_bk")
        nc.sync.dma_start(t_bk[:], dbk)
        t_bva = cst.tile([1, H * DHA], F16, name="t_bva")
        nc.sync.dma_start(t_bva[:], dbva)
        t_bo = cst.tile([1, D], F16, name="t_bo")
        nc.sync.dma_start(t_bo[:], dbo)
        t_Wo = []
        for k in range(4):
            w = cst.tile([128, D], F16, name=f"t_Wo{k}")
            nc.sync.dma_start(w[:], dWo[k * 128:(k + 1) * 128, :])
            t_Wo.append(w)

        # persistent projected tensors
        t_qT = [proj.tile([128, SQ], F16, name=f"qT{m}") for m in range(4)]
        t_kT = [proj.tile([128, S], F16, name=f"kT{m}") for m in range(4)]
        t_v = [proj.tile([128, H * DHA], F16, name=f"v{i}")
               for i in range(NSKT)]

        for rep in range(repeat):
            with tc.tile_pool(name="wstage", bufs=1) as wst, \
                 tc.tile_pool(name="xstage", bufs=2) as xst, \
                 tc.tile_pool(name="pps", bufs=2, space="PSUM") as pps:
                # ---------------- prologue: projections ----------------
                t_Wq = []
                for k in range(4):
                    w = wst.tile([128, D], F16, name=f"t_Wq{k}")
                    nc.sync.dma_start(w[:], dWq[k * 128:(k + 1) * 128, :])
                    t_Wq.append(w)
                t_QT = []
                for k in range(4):
                    q = wst.tile([128, SQ], F16, name=f"t_QT{k}")
                    nc.sync.dma_start(q[:], dQT[k * 128:(k + 1) * 128, :])
                    t_QT.append(q)
                # qT[m] [128, 1024] = sum_k Wq[k][:,m*128:...]^T @ QT[k] + bq
                for m in range(4):
                    pq = pps.tile([128, 1024], F32, name="pq", tag="pp")
                    for half in range(2):
                        o = pq[:, half * 512:(half + 1) * 512]
                        for k in range(4):
                            nc.tensor.matmul(
                                o, t_Wq[k][:, m * 128:(m + 1) * 128],
                                t_QT[k][:, half * 512:(half + 1) * 512],
                                start=(k == 0), stop=False)
                        nc.tensor.matmul(
                            o, t_bq[0:1, m * 128:(m + 1) * 128],
                            t_ones[0:1, 0:512], start=False, stop=True)
                    nc.scalar.copy(t_qT[m][:], pq[:])

                t_Wk = []
                for k in range(4):
                    w = wst.tile([128, D], F16, name=f"t_Wk{k}")
                    nc.sync.dma_start(w[:], dWk[k * 128:(k + 1) * 128, :])
                    t_Wk.append(w)
                # kT[m] [128, 4096], processed in sk chunks of 1024
                for blk in range(4):
                    t_KTc = []
                    for k in range(4):
                        kc = xst.tile([128, 1024], F16, name=f"t_KTc{k}",
                                      tag=f"xk{k}")
                        nc.sync.dma_start(
                            kc[:], dKT[k * 128:(k + 1) * 128,
                                       blk * 1024:(blk + 1) * 1024])
                        t_KTc.append(kc)
                    for m in range(4):
                        pk = pps.tile([128, 1024], F32, name="pk", tag="pp")
                        for half in range(2):
                            o = pk[:, half * 512:(half + 1) * 512]
                            for k in range(4):
                                nc.tensor.matmul(
                                    o, t_Wk[k][:, m * 128:(m + 1) * 128],
                                    t_KTc[k][:, half * 512:(half + 1) * 512],
                                    start=(k == 0), stop=False)
                            nc.tensor.matmul(
                                o, t_bk[0:1, m * 128:(m + 1) * 128],
                                t_ones[0:1, 0:512], start=False, stop=True)
                        nc.scalar.copy(
                            t_kT[m][:, blk * 1024:(blk + 1) * 1024], pk[:])

                t_Wva = []
                for k in range(4):
                    w = wst.tile([128, H * DHA], F16, name=f"t_Wva{k}")
                    nc.sync.dma_start(w[:], dWva[k * 128:(k + 1) * 128, :])
                    t_Wva.append(w)
                # v[skt] [128, 520] = VT[:, skt]^T @ Wva + bva
                for blk in range(4):
                    t_VTc = []
                    for k in range(4):
                        vc = xst.tile([128, 1024], F16, name=f"t_VTc{k}",
                                      tag=f"xk{k}")
                        nc.sync.dma_start(
                            vc[:], dVT[k * 128:(k + 1) * 128,
                                       blk * 1024:(blk + 1) * 1024])
                        t_VTc.append(vc)
                    for sub in range(8):
                        skt = blk * 8 + sub
                        pv = pps.tile([128, 1024], F32, name="pv", tag="pp")
                        for half in range(2):
                            o = pv[:, half * 512:half * 512 + 260]
                            for k in range(4):
                                nc.tensor.matmul(
                                    o, t_VTc[k][:, sub * 128:(sub + 1) * 128],
                                    t_Wva[k][:, half * 260:(half + 1) * 260],
                                    start=(k == 0), stop=False)
                            nc.tensor.matmul(
                                o, t_ones[0:1, 0:128],
                                t_bva[0:1, half * 260:(half + 1) * 260],
                                start=False, stop=True)
                        # strided evac [0:260] + [512:772] -> [128, 520]
                        nc.scalar.copy(
                            t_v[skt][:].rearrange("p (a s) -> p a s", a=2),
                            pv[:].rearrange("p (a s) -> p a s", a=2)
                                 [:, :, 0:260])

            # ---------------- main loop ----------------
            with tc.tile_pool(name="mt", bufs=1) as mtp, \
                 tc.tile_pool(name="em", bufs=4) as emp, \
                 tc.tile_pool(name="er", bufs=3) as erp, \
                 tc.tile_pool(name="pm", bufs=3) as pmp, \
                 tc.tile_pool(name="ctxn", bufs=2) as cxp, \
                 tc.tile_pool(name="sml", bufs=2) as sml, \
                 tc.tile_pool(name="psc", bufs=2, space="PSUM") as psc, \
                 tc.tile_pool(name="pcx", bufs=2, space="PSUM") as pcx, \
                 tc.tile_pool(name="pbc", bufs=1, space="PSUM") as pbc:
                for sqb in range(2):
                    sq0 = sqb * SQB
                    # mask^T tiles for this query block: 4 x [128, 8*512]
                    t_mT = []
                    for g in range(4):
                        mt = mtp.tile([128, 8 * SQB], F16, name=f"mT{g}",
                                      tag=f"mT{g}")
                        nc.sync.dma_start(
                            mt[:].rearrange("p (a s) -> p a s", s=SQB),
                            dMT[g * 1024:(g + 1) * 1024, sq0:sq0 + SQB]
                            .rearrange("(a p) s -> p a s", p=128))
                        t_mT.append(mt)

                    ctxn = [cxp.tile([128, SQB], F16, name=f"cx{dt}",
                                     tag=f"cx{dt}") for dt in range(4)]

                    for h in range(8):
                        ksl = t_kT[h // 2][(h % 2) * 64:(h % 2) * 64 + 64, :]
                        qsl = t_qT[h // 2][(h % 2) * 64:(h % 2) * 64 + 64,
                                           sq0:sq0 + SQB]
                        ctx_ps = pcx.tile([DHA, SQB], F32, name="ctx",
                                          tag="ctx")
                        ems = []
                        for g in range(4):
                            em = emp.tile([128, 8 * SQB], F16, name="em",
                                          tag="em")
                            ems.append(em)
                            for pair in range(4):
                                sc = psc.tile([128, 1024], F32, name="sc",
                                              tag="sc")
                                for j in range(2):
                                    skt = g * 8 + pair * 2 + j
                                    nc.tensor.matmul(
                                        sc[:, j * 512:(j + 1) * 512],
                                        ksl[:, skt * 128:(skt + 1) * 128],
                                        qsl, start=True, stop=True)
                                er = erp.tile([128, 1024], F16, name="er",
                                              tag="er")
                                nc.scalar.activation(er[:], sc[:], EXP)
                                nc.vector.tensor_tensor(
                                    em[:, pair * 1024:(pair + 1) * 1024],
                                    er[:],
                                    t_mT[g][:, pair * 1024:(pair + 1) * 1024],
                                    MULT)
                            for sub in range(8):
                                skt = g * 8 + sub
                                nc.tensor.matmul(
                                    ctx_ps[:],
                                    t_v[skt][:, h * DHA:(h + 1) * DHA],
                                    em[:, sub * SQB:(sub + 1) * SQB],
                                    start=(skt == 0), stop=(skt == NSKT - 1))
                        # rowsum -> reciprocal -> broadcast to 128 partitions
                        rsi = sml.tile([1, SQB], F32, name="rsi", tag="rsi")
                        nc.vector.reciprocal(rsi[:], ctx_ps[64:65, :])
                        rsi16 = sml.tile([1, SQB], F16, name="rsi16",
                                         tag="rsi16")
                        nc.vector.tensor_copy(rsi16[:], rsi[:])
                        bc_ps = pbc.tile([128, SQB], F32, name="bc", tag="bc")
                        nc.tensor.matmul(bc_ps[:], t_ones[0:1, 0:128],
                                         rsi16[:], start=True, stop=True)
                        rbw = sml.tile([128, 2 * SQB], F16, name="rbw",
                                       tag="rbw")
                        nc.scalar.copy(rbw[:, 0:SQB], bc_ps[:])
                        nc.scalar.copy(rbw[:, SQB:2 * SQB], bc_ps[:])
                        # normalized context -> packed ctxn (2 heads / tile)
                        nc.vector.tensor_tensor(
                            ctxn[h // 2][(h % 2) * 64:(h % 2) * 64 + 64, :],
                            ctx_ps[0:64, :], rbw[0:64, 0:SQB], MULT)
                        # P^T = Em * 1/rs -> DRAM (quarters of 2 sk-tiles)
                        for g in range(4):
                            for q in range(4):
                                pm = pmp.tile([128, 2 * SQB], F32, name="pm",
                                              tag="pm")
                                nc.vector.tensor_tensor(
                                    pm[:],
                                    ems[g][:, q * 2 * SQB:(q + 1) * 2 * SQB],
                                    rbw[:], MULT)
                                nc.sync.dma_start(
                                    dAttnT[h,
                                           g * 1024 + q * 256:
                                           g * 1024 + (q + 1) * 256,
                                           sq0:sq0 + SQB]
                                    .rearrange("(a p) s -> p a s", p=128),
                                    pm[:].rearrange("p (a s) -> p a s",
                                                    s=SQB))
                    # output projection for this query block
                    for sqt in range(4):
                        op = pbc.tile([128, D], F32, name="op", tag="bc")
                        for dt in range(4):
                            nc.tensor.matmul(
                                op[:],
                                ctxn[dt][:, sqt * 128:(sqt + 1) * 128],
                                t_Wo[dt][:], start=(dt == 0), stop=False)
                        nc.tensor.matmul(op[:], t_ones[0:1, 0:128],
                                         t_bo[0:1, :], start=False, stop=True)
                        osb = sml.tile([128, D], F32, name="osb", tag="osb")
                        nc.scalar.copy(osb[:], op[:])
                        nc.sync.dma_start(
                            dOut[sq0 + sqt * 128:sq0 + (sqt + 1) * 128, :],
                            osb[:])

    nc.compile()
    _BUILD_CACHE[repeat] = nc
    return nc


def make_in_maps(Q, K, V, attn_mask, Wq, bq, Wk, bk, Wv, bv, Wo, bo):
    f16 = np.float16
    scale = np.float32(1.0 / np.sqrt(DH))
    Wq_s = (np.asarray(Wq, np.float32) * scale).astype(f16)
    bq_s = (np.asarray(bq, np.float32) * scale).astype(f16).reshape(1, D)
    Wk_h = np.asarray(Wk, f16)
    bk_h = np.asarray(bk, f16).reshape(1, D)
    Wva = np.zeros((D, H * DHA), f16)
    bva = np.zeros((1, H * DHA), f16)
    for h in range(H):
        Wva[:, h * DHA:h * DHA + DH] = np.asarray(Wv, f16)[:, h * DH:(h + 1) * DH]
        bva[0, h * DHA:h * DHA + DH] = np.asarray(bv, f16)[h * DH:(h + 1) * DH]
        bva[0, h * DHA + DH] = 1.0
    Wo_h = np.asarray(Wo, f16)
    bo_h = np.asarray(bo, f16).reshape(1, D)
    ones = np.ones((1, SQ), f16)

    in_maps = []
    for c in range(N_CORES):
        b, q0 = c // 4, (c % 4) * SQ
        QT = np.ascontiguousarray(
            np.asarray(Q[b, q0:q0 + SQ, :], np.float32).T).astype(f16)
        KT = np.ascontiguousarray(
            np.asarray(K[b], np.float32).T).astype(f16)
        VT = np.ascontiguousarray(
            np.asarray(V[b], np.float32).T).astype(f16)
        mbarT = np.ascontiguousarray(
            (~np.asarray(attn_mask[b, q0:q0 + SQ, :])).T).astype(f16)
        in_maps.append({
            "QT": QT, "KT": KT, "VT": VT, "mbarT": mbarT,
            "Wq": Wq_s, "Wk": Wk_h, "Wva": Wva, "Wo": Wo_h,
            "bq": bq_s, "bk": bk_h, "bva": bva, "bo": bo_h,
            "ones": ones,
        })
    return in_maps


def kernel(Q, K, V, attn_mask, Wq, bq, Wk, bk, Wv, bv, Wo, bo):
    nc = build(repeat=1)
    in_maps = make_in_maps(Q, K, V, attn_mask, Wq, bq, Wk, bk, Wv, bv, Wo, bo)
    res = run_bass_kernel_spmd(nc, in_maps, list(range(N_CORES)))
    output = np.empty((B, S, D), np.float32)
    attn = np.empty((B, H, S, S), np.float32)
    for c in range(N_CORES):
        b, q0 = c // 4, (c % 4) * SQ
        r = res.results[c]
        output[b, q0:q0 + SQ, :] = r["outp"]
        attn[b, :, q0:q0 + SQ, :] = r["attnT"].transpose(0, 2, 1)
    return output, attn
